# revision 14
# baseline (speedup 1.0000x reference)
"""MiniMax sparse-MoE block on 8 Trainium2 NeuronCores.

Strategy (expert-parallel, per the sharding hint):
  - Router (gates matmul + sigmoid + top-2 + weight normalization) runs on
    host CPU with exactly the reference's jax ops, bit-matching its
    routing decisions.  This *is* the dispatch step: tokens are gathered
    per selected expert ("all-to-all by top-k expert index") while
    building the per-core input shards.
  - Each of the 8 cores owns E/8 = 2 experts.  A core runs the SwitchGLU
    MLP (silu(x@w_gate) * (x@w_up)) @ w_down for the tokens routed to its
    experts only (capacity = max expert load, rounded up), with weights
    stationary on the PE array and tokens as the moving operand
    (activations kept transposed: [H, tokens]).
  - Matmuls run in float32r mode (full-rate fp32 PE path; the plain fp32
    path is 4x slower).  PSUM accumulation is fp32.
  - Host combines: y[t] = sum over the token's 2 experts of
    sel_weight * expert_out — two fp32 terms, order-independent.
"""

import os
import sys
import functools

for _p in ("/opt/trn_rl_repo", "/root/.axon_site/_ro/trn_rl_repo"):
    if os.path.isdir(_p) and _p not in sys.path:
        sys.path.append(_p)

import numpy as np

T, H, F, E, KTOP = 2048, 1024, 1024, 16, 2
NCORES = 8
EPC = E // NCORES  # experts per core
P = 128
KO = H // P  # contraction chunks per 1024-dim
FB = F // P  # 128-blocks of F
HB = H // P  # 128-blocks of H

# "f16"  = fp16 operands (half the weight DMA bytes, full-rate PE,
#          ~2e-4 rel err per matmul from operand quantization)
# "f32r" = float32r single-pass PE mode (~1.5e-4 rel err per matmul)
# "f32"  = exact fp32 PE mode (4x slower)
MM_MODE = os.environ.get("MOE_MM_MODE", "f16")

LAST_RESULTS = None  # BassKernelResults of the most recent device run


def _chunks(cap):
    """Split cap into moving-dim chunks <= 512 (PSUM bank / fp32 AP limit)."""
    out, rem, n = [], cap, -(-cap // 512)
    for i in range(n):
        c = min(512, rem, -(-rem // ((n - i) * 64)) * 64)
        out.append(c)
        rem -= c
    assert sum(out) == cap and all(0 < c <= 512 for c in out), (cap, out)
    return out


@functools.lru_cache(maxsize=4)
def _build_program(cap):
    import concourse.mybir as mybir
    import concourse.tile as tile
    from concourse import bacc

    f32 = mybir.dt.float32
    mm_dt = {"f16": mybir.dt.float16,
             "f32r": mybir.dt.float32r,
             "f32": f32}[MM_MODE]
    silu = mybir.ActivationFunctionType.Silu

    nc = bacc.Bacc("TRN2", target_bir_lowering=False, debug=False,
                   num_devices=NCORES)

    xt_d, wg_d, wu_d, wd_d, yt_d = [], [], [], [], []
    for s in range(EPC):
        xt_d.append(nc.dram_tensor(f"xt{s}", [H, cap], mm_dt,
                                   kind="ExternalInput").ap())
        wg_d.append(nc.dram_tensor(f"wg{s}", [FB, P, H], mm_dt,
                                   kind="ExternalInput").ap())
        wu_d.append(nc.dram_tensor(f"wu{s}", [FB, P, H], mm_dt,
                                   kind="ExternalInput").ap())
        wd_d.append(nc.dram_tensor(f"wd{s}", [HB, P, F], mm_dt,
                                   kind="ExternalInput").ap())
        yt_d.append(nc.dram_tensor(f"yt{s}", [HB, P, cap], f32,
                                   kind="ExternalOutput").ap())

    cols = _chunks(cap)
    col_off = [0]
    for c in cols:
        col_off.append(col_off[-1] + c)

    def mm(ps, lhsT, rhs, start, stop):
        nc.tensor.matmul(ps, lhsT=lhsT, rhs=rhs, start=start, stop=stop)

    with tile.TileContext(nc) as tc:
        with (
            tc.tile_pool(name="xp", bufs=2) as xp,
            tc.tile_pool(name="wp", bufs=32) as wp,
            tc.tile_pool(name="sp", bufs=6) as sp,
            tc.tile_pool(name="hp", bufs=2) as hp,
            tc.tile_pool(name="op", bufs=6) as op,
            tc.tile_pool(name="pp", bufs=8, space="PSUM") as pp,
        ):
            for s in range(EPC):
                xt = xp.tile([P, KO, cap], mm_dt, tag="xt")
                nc.sync.dma_start(xt, xt_d[s].rearrange("(ko p) n -> p ko n", p=P))
                h_sb = hp.tile([P, FB, cap], mm_dt, tag="h")
                for f in range(FB):
                    wgf = wp.tile([P, KO, P], mm_dt, tag="w")
                    nc.sync.dma_start(wgf, wg_d[s][f].rearrange("p (ko m) -> p ko m", m=P))
                    wuf = wp.tile([P, KO, P], mm_dt, tag="w")
                    nc.sync.dma_start(wuf, wu_d[s][f].rearrange("p (ko m) -> p ko m", m=P))
                    for ci, ncol in enumerate(cols):
                        c0, c1 = col_off[ci], col_off[ci + 1]
                        psg = pp.tile([P, ncol], f32, tag=f"ps{ncol}")
                        psu = pp.tile([P, ncol], f32, tag=f"ps{ncol}")
                        for k in range(KO):
                            mm(psg, wgf[:, k], xt[:, k, c0:c1], k == 0, k == KO - 1)
                        for k in range(KO):
                            mm(psu, wuf[:, k], xt[:, k, c0:c1], k == 0, k == KO - 1)
                        sg = sp.tile([P, ncol], f32, tag=f"sg{ncol}")
                        nc.scalar.activation(sg, psg, silu)
                        nc.vector.tensor_mul(out=h_sb[:, f, c0:c1], in0=sg, in1=psu)
                for hb in range(HB):
                    wdf = wp.tile([P, FB, P], mm_dt, tag="w")
                    nc.sync.dma_start(wdf, wd_d[s][hb].rearrange("p (fb m) -> p fb m", m=P))
                    for ci, ncol in enumerate(cols):
                        c0, c1 = col_off[ci], col_off[ci + 1]
                        psy = pp.tile([P, ncol], f32, tag=f"ps{ncol}")
                        for f in range(FB):
                            mm(psy, wdf[:, f], h_sb[:, f, c0:c1], f == 0, f == FB - 1)
                        ysb = op.tile([P, ncol], f32, tag=f"y{ncol}")
                        nc.vector.tensor_copy(out=ysb, in_=psy)
                        nc.sync.dma_start(yt_d[s][hb, :, c0:c1], ysb)

    nc.compile()
    return nc


def _route(x, gate_w, bias):
    """Top-2 routing with exactly the reference's jax ops on CPU."""
    import jax
    import jax.numpy as jnp

    cpu = jax.devices("cpu")[0]
    with jax.default_device(cpu):
        xd = jax.device_put(x, cpu)
        gd = jax.device_put(gate_w, cpu)
        bd = jax.device_put(bias, cpu)
        gates = jnp.einsum("th,eh->te", xd.astype(jnp.float32), gd)
        orig = jax.nn.sigmoid(gates)
        corrected = orig + bd
        _, inds = jax.lax.top_k(corrected, KTOP)
        sel = jnp.take_along_axis(orig, inds, axis=-1)
        sel = sel / (jnp.sum(sel, axis=-1, keepdims=True) + 1e-20)
        sel = sel.astype(x.dtype)
    return np.asarray(inds), np.asarray(sel)


_PACK_CACHE = {}


NP_MM_DT = np.float16 if MM_MODE == "f16" else np.float32


def _pack(w):
    """[1024, 1024] -> [8, 128, 1024] blocks: out[b, p, k*128+m] = w[k*128+p, b*128+m]."""
    return np.ascontiguousarray(
        w.reshape(8, P, 8, P).transpose(2, 1, 0, 3).reshape(8, P, 8 * P)
        .astype(NP_MM_DT))


def kernel(x, gate_w, w_gate, w_up, w_down, e_score_correction_bias):
    global LAST_RESULTS
    from concourse import bass_utils

    x = np.asarray(x, dtype=np.float32)
    inds, sel = _route(x, np.asarray(gate_w, np.float32),
                       np.asarray(e_score_correction_bias, np.float32))

    # dispatch: token lists per expert
    tok_idx, tok_w = [], []
    maxcnt = 1
    for e in range(E):
        rows, slots = np.nonzero(inds == e)
        tok_idx.append(rows)
        tok_w.append(sel[rows, slots])
        maxcnt = max(maxcnt, len(rows))
    if MM_MODE == "f16":
        # fp16 matmul is full-rate at any moving dim; just align to 16
        cap = max(64, -(-maxcnt // 16) * 16)
    else:
        # float32r needs moving dim >= 256 for the full-rate PE path
        cap = max(256, -(-maxcnt // 64) * 64)

    nc = _build_program(cap)

    # weight packing (cached on the weight buffers' identity)
    wkey = (id(w_gate), id(w_up), id(w_down),
            w_gate.shape if hasattr(w_gate, "shape") else None)
    packed = _PACK_CACHE.get(wkey)
    if packed is None:
        wg = np.asarray(w_gate, np.float32)
        wu = np.asarray(w_up, np.float32)
        wd = np.asarray(w_down, np.float32)
        packed = ([_pack(wg[e]) for e in range(E)],
                  [_pack(wu[e]) for e in range(E)],
                  [_pack(wd[e]) for e in range(E)])
        _PACK_CACHE.clear()
        _PACK_CACHE[wkey] = packed
    wg_p, wu_p, wd_p = packed

    in_maps = []
    for c in range(NCORES):
        m = {}
        for s in range(EPC):
            e = c * EPC + s
            xt = np.zeros((H, cap), NP_MM_DT)
            cnt = len(tok_idx[e])
            if cnt:
                xt[:, :cnt] = x[tok_idx[e]].T.astype(NP_MM_DT)
            m[f"xt{s}"] = xt
            m[f"wg{s}"] = wg_p[e]
            m[f"wu{s}"] = wu_p[e]
            m[f"wd{s}"] = wd_p[e]
        in_maps.append(m)

    res = bass_utils.run_bass_kernel_spmd(nc, in_maps, core_ids=list(range(NCORES)))
    LAST_RESULTS = res

    y = np.zeros((T, H), np.float32)
    for c in range(NCORES):
        for s in range(EPC):
            e = c * EPC + s
            cnt = len(tok_idx[e])
            if not cnt:
                continue
            yt = res.results[c][f"yt{s}"].reshape(H, cap)
            y[tok_idx[e]] += tok_w[e][:, None] * yt[:, :cnt].T
    return y
